# revision 15
# baseline (speedup 1.0000x reference)
"""CoordinatesToSpikes on 8 TRN2 NeuronCores.

Reference semantics: times = T_EARLY + cv * (T_LATE - T_EARLY);
idx = round(times / DT); spikes = one-hot along a dense time axis of
length 1000 (each (b, c) pair scatters exactly one 1.0, so the scatter
is a pure one-hot materialization: out[b, t, c] = (idx[b, c] == t)).

Strategy (data-parallel over batch, 256 -> 8 x 32):
  - Host computes idx bit-exactly in fp32 (tiny: 64K elements) and a
    per-core diff tensor diff[p, f] = idx[p//4, f%256] - (p%4)*250
    - f//256 (1.25MB/core). All values are exact small integers.
  - On device, SBUF partition p covers batch b = p//4, time-quarter
    tg = p%4 (250 time rows each) so every partition's slice of the
    output is one contiguous 250KB DRAM range -> 10KB DMA descriptors
    (1KB descriptors cap a single HWDGE ring at ~115 GB/s; 10KB ones
    run at full SDMA rate).
  - Each of 25 chunks (10 time rows) is one DVE compare diff == 10*d
    producing the one-hot tile [128, 2560], DMA-stored as a 1.25MB
    transfer with 10KB contiguous per partition, alternating between
    the two HWDGE rings (sync + scalar engines).
  - Output is write-only, 32.8 MB per core => memory(store)-roofline
    (~91us per core; HBM stacks are shared pairwise, 716 GB/s per
    2 cores, so ~358 GB/s/core sustained).
"""

import numpy as np
from contextlib import ExitStack

import concourse.bass as bass
import concourse.tile as tile
from concourse import bacc, mybir
from concourse.bass_utils import run_bass_kernel_spmd

F32 = mybir.dt.float32

B, C, SEQ = 256, 256, 1000
NCORES = 8
BSH = B // NCORES          # 32 batches per core
TG = 4                     # time quarters per batch (partition = b*4+tg)
TQ = SEQ // TG             # 250 time rows per quarter
TROWS = 10                 # time rows per chunk
ND = TQ // TROWS           # 25 chunks
FREE = TROWS * C           # 2560 free elements per tile (10KB)

T_EARLY = np.float32(2e-06)
T_LATE_MINUS_EARLY = np.float32(0.0008 - 2e-06)
DT = np.float32(1e-06)

_compiled = None


def _build():
    nc = bacc.Bacc("TRN2", target_bir_lowering=False, debug=False,
                   num_devices=NCORES)
    diff_d = nc.dram_tensor("diff", [128, FREE], F32, kind="ExternalInput")
    out_d = nc.dram_tensor("out", [BSH, SEQ, C], F32, kind="ExternalOutput")
    # [128 partitions (b,tg) @ 250KB stride, 25 chunks, 2560 contiguous]
    out_v = out_d.ap().rearrange(
        "b (tg d t) c -> (b tg) d (t c)", tg=TG, d=ND, t=TROWS)

    quart = FREE // 4
    with ExitStack() as ctx:
        tc = ctx.enter_context(tile.TileContext(nc))
        dpool = ctx.enter_context(tc.tile_pool(name="diff", bufs=1))
        outp = ctx.enter_context(tc.tile_pool(name="outp", bufs=10))

        # Load diff in four quarters spread over three DGE queues so the
        # first chunk-0 piece can start as early as possible.
        engines = [nc.sync, nc.scalar, nc.gpsimd]
        diff = dpool.tile([128, FREE], F32)
        for q in range(4):
            engines[q % 3].dma_start(
                diff[:, q * quart:(q + 1) * quart],
                diff_d.ap()[:, q * quart:(q + 1) * quart])

        # Chunk 0 is computed/stored as four column pieces, each gated
        # only on its own quarter of the load (column slices of the
        # chunk stay contiguous per partition in DRAM); remaining chunks
        # go full-width. Stores rotate across the three DGE queues.
        for q in range(4):
            oq = outp.tile([128, quart], F32, tag="piece")
            nc.vector.tensor_scalar(
                oq[:], diff[:, q * quart:(q + 1) * quart], 0.0, None,
                mybir.AluOpType.is_equal)
            engines[q % 3].dma_start(
                out_v[:, 0, q * quart:(q + 1) * quart], oq[:])

        for d in range(1, ND):
            ot = outp.tile([128, FREE], F32)
            nc.vector.tensor_scalar(
                ot[:], diff[:], float(TROWS * d), None,
                mybir.AluOpType.is_equal)
            engines[d % 3].dma_start(out_v[:, d, :], ot[:])
    nc.compile()
    return nc


def _host_idx(coordinate_values: np.ndarray) -> np.ndarray:
    """Bit-exact fp32 mirror of the reference index computation."""
    cv = np.ascontiguousarray(coordinate_values, dtype=np.float32)
    times = T_EARLY + cv * T_LATE_MINUS_EARLY
    return np.rint(times / DT).astype(np.float32)


def _in_maps(coordinate_values: np.ndarray) -> list[dict]:
    idxf = _host_idx(coordinate_values)                      # (256, 256)
    p = np.arange(128)
    base = ((p % TG) * TQ)[:, None] + np.repeat(
        np.arange(TROWS), C)[None, :]                        # (128, 2560)
    maps = []
    for m in range(NCORES):
        shard = idxf[m * BSH:(m + 1) * BSH]                  # (32, 256)
        tiled = np.tile(shard[p // TG], (1, TROWS))          # (128, 2560)
        maps.append({"diff": (tiled - base).astype(np.float32)})
    return maps


def kernel(coordinate_values: np.ndarray) -> np.ndarray:
    global _compiled
    if _compiled is None:
        _compiled = _build()
    res = run_bass_kernel_spmd(
        _compiled, _in_maps(coordinate_values),
        core_ids=list(range(NCORES)))
    return np.concatenate([r["out"] for r in res.results], axis=0)


# revision 16
# speedup vs baseline: 1.0221x; 1.0221x over previous
"""CoordinatesToSpikes on 8 TRN2 NeuronCores.

Reference semantics: times = T_EARLY + cv * (T_LATE - T_EARLY);
idx = round(times / DT); spikes = one-hot along a dense time axis of
length 1000 (each (b, c) pair scatters exactly one 1.0, so the scatter
is a pure one-hot materialization: out[b, t, c] = (idx[b, c] == t)).

Strategy (data-parallel over batch, 256 -> 8 x 32):
  - Host computes idx bit-exactly in fp32 (tiny: 64K elements) and a
    per-core diff tensor diff[p, f] = idx[p//4, f%256] - (p%4)*250
    - f//256 (1.25MB/core). All values are exact small integers.
  - On device, SBUF partition p covers batch b = p//4, time-quarter
    tg = p%4 (250 time rows each) so every partition's slice of the
    output is one contiguous 250KB DRAM range -> 10KB DMA descriptors
    (1KB descriptors cap a single HWDGE ring at ~115 GB/s; 10KB ones
    run at full SDMA rate).
  - Each of 25 chunks (10 time rows) is one DVE compare diff == 10*d
    producing the one-hot tile [128, 2560], DMA-stored as a 1.25MB
    transfer with 10KB contiguous per partition, alternating between
    the two HWDGE rings (sync + scalar engines).
  - Output is write-only, 32.8 MB per core => memory(store)-roofline
    (~91us per core; HBM stacks are shared pairwise, 716 GB/s per
    2 cores, so ~358 GB/s/core sustained).
"""

import numpy as np
from contextlib import ExitStack

import concourse.bass as bass
import concourse.tile as tile
from concourse import bacc, mybir
from concourse.bass_utils import run_bass_kernel_spmd

F32 = mybir.dt.float32

B, C, SEQ = 256, 256, 1000
NCORES = 8
BSH = B // NCORES          # 32 batches per core
TG = 4                     # time quarters per batch (partition = b*4+tg)
TQ = SEQ // TG             # 250 time rows per quarter
TROWS = 10                 # time rows per chunk
ND = TQ // TROWS           # 25 chunks
FREE = TROWS * C           # 2560 free elements per tile (10KB)

T_EARLY = np.float32(2e-06)
T_LATE_MINUS_EARLY = np.float32(0.0008 - 2e-06)
DT = np.float32(1e-06)

_compiled = None


def _build():
    nc = bacc.Bacc("TRN2", target_bir_lowering=False, debug=False,
                   num_devices=NCORES)
    diff_d = nc.dram_tensor("diff", [128, FREE], F32, kind="ExternalInput")
    out_d = nc.dram_tensor("out", [BSH, SEQ, C], F32, kind="ExternalOutput")
    # [128 partitions (b,tg) @ 250KB stride, 25 chunks, 2560 contiguous]
    out_v = out_d.ap().rearrange(
        "b (tg d t) c -> (b tg) d (t c)", tg=TG, d=ND, t=TROWS)

    quart = FREE // 4
    with ExitStack() as ctx:
        tc = ctx.enter_context(tile.TileContext(nc))
        dpool = ctx.enter_context(tc.tile_pool(name="diff", bufs=1))
        outp = ctx.enter_context(tc.tile_pool(name="outp", bufs=10))

        # Load diff in four quarters, two per HWDGE ring (the gpsimd
        # SWDGE ring has ~1us extra first-byte latency — stores only),
        # so the first chunk-0 piece can start as early as possible.
        engines = [nc.sync, nc.scalar, nc.gpsimd]
        diff = dpool.tile([128, FREE], F32)
        for q in range(4):
            engines[q % 2].dma_start(
                diff[:, q * quart:(q + 1) * quart],
                diff_d.ap()[:, q * quart:(q + 1) * quart])

        # Chunk 0 is computed/stored as four column pieces, each gated
        # only on its own quarter of the load (column slices of the
        # chunk stay contiguous per partition in DRAM); remaining chunks
        # go full-width. Stores rotate across the three DGE queues.
        for q in range(4):
            oq = outp.tile([128, quart], F32, tag="piece")
            nc.vector.tensor_scalar(
                oq[:], diff[:, q * quart:(q + 1) * quart], 0.0, None,
                mybir.AluOpType.is_equal)
            engines[q % 3].dma_start(
                out_v[:, 0, q * quart:(q + 1) * quart], oq[:])

        for d in range(1, ND):
            ot = outp.tile([128, FREE], F32)
            nc.vector.tensor_scalar(
                ot[:], diff[:], float(TROWS * d), None,
                mybir.AluOpType.is_equal)
            engines[d % 3].dma_start(out_v[:, d, :], ot[:])
    nc.compile()
    return nc


def _host_idx(coordinate_values: np.ndarray) -> np.ndarray:
    """Bit-exact fp32 mirror of the reference index computation."""
    cv = np.ascontiguousarray(coordinate_values, dtype=np.float32)
    times = T_EARLY + cv * T_LATE_MINUS_EARLY
    return np.rint(times / DT).astype(np.float32)


def _in_maps(coordinate_values: np.ndarray) -> list[dict]:
    idxf = _host_idx(coordinate_values)                      # (256, 256)
    p = np.arange(128)
    base = ((p % TG) * TQ)[:, None] + np.repeat(
        np.arange(TROWS), C)[None, :]                        # (128, 2560)
    maps = []
    for m in range(NCORES):
        shard = idxf[m * BSH:(m + 1) * BSH]                  # (32, 256)
        tiled = np.tile(shard[p // TG], (1, TROWS))          # (128, 2560)
        maps.append({"diff": (tiled - base).astype(np.float32)})
    return maps


def kernel(coordinate_values: np.ndarray) -> np.ndarray:
    global _compiled
    if _compiled is None:
        _compiled = _build()
    res = run_bass_kernel_spmd(
        _compiled, _in_maps(coordinate_values),
        core_ids=list(range(NCORES)))
    return np.concatenate([r["out"] for r in res.results], axis=0)
